# revision 24
# baseline (speedup 1.0000x reference)
"""Trainium2 Bass kernel for BaseSegHead (dynamic 1x1-conv seg logits).

Computes, for full inputs:
    qry_feats = in_feats @ qry_w.T + qry_b                  [1200, 32]
    key_map   = einsum('oc,bchw->bohw', key_w, feat_map) + key_b
    logits    = einsum('bnc,bchw->bnhw', qry_feats.reshape(4,300,32), key_map)
    out       = logits.reshape(1200, 160, 160)

Sharding: 8 cores = 4 batch images x 2 spatial (H) halves. Core c handles
batch b = c//2, rows h*80:(h+1)*80. Each core reads feat_map[b,:,rows,:],
its 300 queries, and writes a [300, 80*160] output shard -- no cross-core
communication and no duplicated feat_map reads.

Precision: matmul operands are shipped/produced as fp16 (full-rate on the
PE array; halves DMA bytes); accumulation stays fp32 in PSUM. The fp32
logits are rounded to fp16 for the output DMA and upcast on the host.

v4 layout (trace-driven): the kernel is HBM-bound (~14.4 MB of traffic).
All data DMAs ride the sync HW-DGE ring in readiness order.  feat_map is
host-packed so each of 7 input triggers delivers one 2048-column block
with BOTH channel halves (8 KB/partition rows): the first key quad can
start ~1 us after the first block lands, and quad k's operands arrive
while quad k-1's output drains.  Output is staged in three full-row SBUF
buffers [*, 12800]; 18 sixth-granularity out DMAs (4 KB rows) fire as
their drains complete, keeping the ring fed from ~15 us on.  PSUM drains
are single-bank [*, 512] copies (6 main PSUM buffers -- pipeline depth
hides the cross-engine semaphore round-trip) assigned to scalar/vector
by accumulated-cost balance; warm-up matmuls un-throttle the PE HAM
clock gate before the first real matmul.

TensorE array tiling: the key projection (M=32) runs 4-way column-tiled
into one PSUM bank per quad of hw-tiles; one bias-activation drains four
tiles. The main einsum (K=32) runs 4-way row-tiled: hw-tile t keeps its
q and key_map operands on SBUF partitions 32*(t%4), so consecutive tiles
issue to distinct PE row-groups and overlap on the array.
"""

import os
import sys

sys.path.insert(0, "/opt/trn_rl_repo")
os.environ.setdefault("MYCRO_LOCAL_CACHE", "1")

import numpy as np

BATCH = 4
N_PER = 300
IN_DIM = 256
KEY_DIM = 32
FH = FW = 160
HHALF = FH // 2            # 80 rows per core
HW = HHALF * FW            # 12800 spatial positions per core
N_CORES = 8

MMN = 512                  # matmul moving free size (one fp32 PSUM bank)
N_T = HW // MMN            # 25 hw-tiles
N_BLK = 6                  # six full 2048-col blocks (quads) + one 512 tail
BLKW = 4 * MMN             # 2048 feat columns per block
# out-DMA groups: tail tile first (it drains first), then 2048-col sixths;
# the last block goes out per-512-tile so the final flush is small.
OUT_GROUPS = ((24 * 512, HW),) + tuple(
    (g * 2048, (g + 1) * 2048) for g in range(5)) + tuple(
    (c, c + 512) for c in range(10240, 12288, 512))

N_CHUNKS = ((0, 128), (128, 128), (256, 44))   # query-row chunks (300 rows)
CPACK_W = 728              # fp16: qry_wT (64) + in_featsT (600) + key_wT (64)
FPW = 2 * HW               # featP width: d0|d1 interleaved per block

_CACHE = {}


def build_nc():
    import concourse.bass as bass
    import concourse.bacc as bacc
    import concourse.mybir as mybir
    from concourse import tile

    f32 = mybir.dt.float32
    f16 = mybir.dt.float16
    Ident = mybir.ActivationFunctionType.Identity

    nc = bacc.Bacc("TRN2", target_bir_lowering=False, debug=False)

    featP = nc.dram_tensor("featP", [128, FPW], f16, kind="ExternalInput")
    cpack = nc.dram_tensor("cpack", [128, CPACK_W], f16, kind="ExternalInput")
    bpack = nc.dram_tensor("bpack", [128, 2], f32, kind="ExternalInput")
    out = nc.dram_tensor("out", [128, 3 * HW], f16, kind="ExternalOutput")

    with tile.TileContext(nc) as tc:
        with (
            tc.tile_pool(name="const", bufs=1) as cpool,
            tc.tile_pool(name="fpool", bufs=1) as fpool,
            tc.tile_pool(name="opool", bufs=1) as opool,
            tc.tile_pool(name="kmap", bufs=1) as kpool,
            tc.tile_pool(name="ps_main", bufs=6, space=bass.MemorySpace.PSUM) as ps_main,
            tc.tile_pool(name="ps_small", bufs=2, space=bass.MemorySpace.PSUM) as ps_small,
        ):
            # --- DMA ring head: consts, then the 7 paired feat blocks ----
            ct = cpool.tile([128, CPACK_W], f16, name="ct")
            nc.sync.dma_start(ct[:], cpack[:])
            bt = cpool.tile([128, 2], f32, name="bt")
            nc.sync.dma_start(bt[:], bpack[:])
            qw = (ct[:, 0:32], ct[:, 32:64])
            inT = (ct[:, 64:364], ct[:, 364:664])
            kw = (ct[:, 664:696], ct[:, 696:728])
            qb = bt[:, 0:1]        # qry_b replicated in all four bands
            kb = bt[:, 1:2]        # key_b replicated in all four bands

            # featP block k: cols [2k*BLKW, (2k+2)*BLKW) = d0 block | d1 block
            # The tiny tail block loads FIRST: it lands ~1us before block 0
            # and primes the whole matmul->drain->out pipeline.
            fp = fpool.tile([128, FPW], f16, name="fp")
            nc.sync.dma_start(fp[:, 2 * N_BLK * BLKW:FPW],
                              featP[:, 2 * N_BLK * BLKW:FPW])
            for k in range(N_BLK):
                nc.sync.dma_start(
                    fp[:, 2 * k * BLKW:2 * (k + 1) * BLKW],
                    featP[:, 2 * k * BLKW:2 * (k + 1) * BLKW],
                )

            def feat(d, t):
                # hw-tile t, channel half d -> fp column range
                k = t // 4
                if k < N_BLK:
                    c0 = 2 * k * BLKW + d * BLKW + (t % 4) * MMN
                else:
                    c0 = 2 * N_BLK * BLKW + d * MMN
                return fp[:, c0:c0 + MMN]

            # Preload the scalar-engine activation table with a dummy op at
            # t~0 (vector memsets a scratch tile first) so the ~1.3us
            # ACT_TABLE_LOAD doesn't delay the first real activation.
            warm = cpool.tile([128, 8], f32, name="warm")
            nc.vector.memset(warm[:], 0.0)
            warm16 = cpool.tile([128, 8], f16, name="warm16")
            nc.scalar.activation(warm16[:], warm[:, 0:8], Ident, bias=warm[:, 0:1])

            # --- qry projection, 4-way column-tiled (4 band copies) -------
            qp = ps_small.tile([128, MMN], f32, name="qp", tag="kp")
            for b in range(4):
                for d in range(2):
                    nc.tensor.matmul(
                        qp[32 * b:32 * b + 32, 0:N_PER],
                        qw[d],
                        inT[d],
                        start=(d == 0),
                        stop=(d == 1),
                        tile_position=(0, 32 * b),
                    )
            q_sb = cpool.tile([128, N_PER], f16, name="q_sb")
            nc.scalar.activation(q_sb[:], qp[:, 0:N_PER], Ident, bias=qb)

            # --- key_map: 4-way column-tiled, banded layout ---------------
            # hw-tile t lives on SBUF partitions 32*(t%4), columns
            # (t//4)*512; one [128,512] PSUM bank holds a whole quad and is
            # drained by a single bias-activation.
            key_map = kpool.tile([128, 7 * MMN], f16, name="key_map")

            # Drains and key-quad bias-adds are assigned to scalar/vector by
            # accumulated-cost balance (vector does bias via
            # tensor_scalar_add) so neither stream head-of-line blocks.
            acc = {"s": 0.0, "v": 0.0}

            def drain(dst, src):
                if acc["s"] + 720 < acc["v"] + 678:
                    nc.scalar.copy(dst, src)
                    acc["s"] += 720
                else:
                    nc.vector.tensor_copy(dst, src)
                    acc["v"] += 678

            def bias_add(dst, src, bias):
                if acc["s"] + 686 < acc["v"] + 678:
                    nc.scalar.activation(dst, src, Ident, bias=bias)
                    acc["s"] += 686
                else:
                    nc.vector.tensor_scalar_add(dst, src, bias)
                    acc["v"] += 678

            def key_quad(k):
                kp = ps_small.tile([128, MMN], f32, name=f"kp_{k}", tag="kp")
                nb = min(4, N_T - 4 * k)
                # d outer, band inner: each round's four column-group
                # matmuls overlap on the PE array.
                for d in range(2):
                    for b in range(nb):
                        nc.tensor.matmul(
                            kp[32 * b:32 * b + 32, :],
                            kw[d],
                            feat(d, 4 * k + b),
                            start=(d == 0),
                            stop=(d == 1),
                            tile_position=(0, 32 * b),
                        )
                p = 32 * nb
                bias_add(key_map[0:p, k * MMN:(k + 1) * MMN], kp[0:p, :],
                         kb[0:p, :])

            # --- output row-buffers: one [*, 12800] tile per query chunk --
            OB = [opool.tile([128, HW], f16, name=f"ob_{j}") for j in range(3)]

            # --- main einsum: 4-way row-tiled over band b = t%4 -----------
            # chunk-outer / tile-inner order: adjacent matmuls target
            # different PE row-groups and overlap on the array.
            def main_tiles(j, tiles):
                n0, m = N_CHUNKS[j]
                for t in tiles:
                    b = t % 4
                    kcol = (t // 4) * MMN
                    mp = ps_main.tile([128, MMN], f32, name=f"mp_{t}_{n0}", tag="mp")
                    nc.tensor.matmul(
                        mp[:m, :],
                        q_sb[32 * b:32 * b + 32, n0:n0 + m],
                        key_map[32 * b:32 * b + 32, kcol:kcol + MMN],
                        tile_position=(32 * b, 0),
                    )
                    drain(OB[j][:m, t * MMN:(t + 1) * MMN], mp[:m, :])

            # Interleave: tail quad/tile first (its data lands first), then
            # each key quad feeds its four hw-tiles.  Quads are emitted one
            # block AHEAD of their main tiles so the tensor stream never
            # stalls waiting for the current block's bias-add: while
            # bias_add(k) pends, the PE runs quad k+1's matmuls.
            key_quad(6)
            for j in range(3):
                main_tiles(j, (24,))
            key_quad(0)
            for k in range(N_BLK):
                if k + 1 < N_BLK:
                    key_quad(k + 1)
                for j in range(3):
                    main_tiles(j, range(4 * k, 4 * k + 4))

            # --- out DMAs: sixth-granularity (4KB rows), readiness order --
            for (c0, c1) in OUT_GROUPS:
                for j, (n0, m) in enumerate(N_CHUNKS):
                    nc.sync.dma_start(
                        out[0:m, j * HW + c0:j * HW + c1], OB[j][0:m, c0:c1]
                    )

    nc.compile()
    return nc


def _get_nc():
    if "nc" not in _CACHE:
        _CACHE["nc"] = build_nc()
    return _CACHE["nc"]


def make_in_maps(in_feats, feat_map, qry_w, qry_b, key_b, key_w):
    qwT = qry_w.T.astype(np.float16)                          # [256, 32]
    kwT = key_w.T.astype(np.float16)                          # [256, 32]
    bpack = np.zeros((128, 2), np.float32)
    bpack[:, 0] = np.tile(qry_b, 4)
    bpack[:, 1] = np.tile(key_b, 4)
    in_maps = []
    for c in range(N_CORES):
        b, h = divmod(c, 2)
        ifT = in_feats[b * N_PER:(b + 1) * N_PER].T.astype(np.float16)
        cpack = np.zeros((128, CPACK_W), np.float16)
        cpack[:, 0:32] = qwT[0:128]
        cpack[:, 32:64] = qwT[128:256]
        cpack[:, 64:364] = ifT[0:128]
        cpack[:, 364:664] = ifT[128:256]
        cpack[:, 664:696] = kwT[0:128]
        cpack[:, 696:728] = kwT[128:256]
        feat16 = np.ascontiguousarray(
            feat_map[b, :, h * HHALF:(h + 1) * HHALF, :]
        ).reshape(IN_DIM, HW).astype(np.float16)
        # featP: block k holds cols [2k*BLKW,(2k+2)*BLKW) = d0 cols | d1 cols
        featP = np.empty((128, FPW), np.float16)
        for k in range(N_BLK + 1):
            w = BLKW if k < N_BLK else MMN
            c0 = k * BLKW
            for d in range(2):
                featP[:, 2 * c0 + d * w:2 * c0 + (d + 1) * w] = (
                    feat16[d * 128:(d + 1) * 128, c0:c0 + w]
                )
        in_maps.append({
            "featP": featP,
            "cpack": cpack,
            "bpack": bpack,
        })
    return in_maps


def kernel(**inputs):
    in_feats = np.asarray(inputs["in_feats"], dtype=np.float32)
    feat_map = np.asarray(inputs["feat_map"], dtype=np.float32)
    qry_w = np.asarray(inputs["qry_w"], dtype=np.float32)
    qry_b = np.asarray(inputs["qry_b"], dtype=np.float32)
    key_w = np.asarray(inputs["key_w"], dtype=np.float32)
    key_b = np.asarray(inputs["key_b"], dtype=np.float32)

    from concourse import bass_utils

    nc = _get_nc()
    in_maps = make_in_maps(in_feats, feat_map, qry_w, qry_b, key_b, key_w)
    trace = os.environ.get("SEG_KERNEL_TRACE", "0") == "1"
    res = bass_utils.run_bass_kernel_spmd(
        nc, in_maps, core_ids=list(range(N_CORES)), trace=trace
    )
    _CACHE["last_result"] = res

    out = np.empty((BATCH * N_PER, FH, FW), dtype=np.float32)
    for c in range(N_CORES):
        b, h = divmod(c, 2)
        raw = res.results[c]["out"].astype(np.float32)        # [128, 3*HW]
        shard = np.empty((N_PER, HW), dtype=np.float32)
        for j, (n0, m) in enumerate(N_CHUNKS):
            shard[n0:n0 + m] = raw[0:m, j * HW:(j + 1) * HW]
        out[b * N_PER:(b + 1) * N_PER, h * HHALF:(h + 1) * HHALF, :] = (
            shard.reshape(N_PER, HHALF, FW)
        )
    return out


# revision 26
# speedup vs baseline: 1.0162x; 1.0162x over previous
"""Trainium2 Bass kernel for BaseSegHead (dynamic 1x1-conv seg logits).

Computes, for full inputs:
    qry_feats = in_feats @ qry_w.T + qry_b                  [1200, 32]
    key_map   = einsum('oc,bchw->bohw', key_w, feat_map) + key_b
    logits    = einsum('bnc,bchw->bnhw', qry_feats.reshape(4,300,32), key_map)
    out       = logits.reshape(1200, 160, 160)

Sharding: 8 cores = 4 batch images x 2 spatial (H) halves. Core c handles
batch b = c//2, rows h*80:(h+1)*80. Each core reads feat_map[b,:,rows,:],
its 300 queries, and writes a [300, 80*160] output shard -- no cross-core
communication and no duplicated feat_map reads.

Precision: matmul operands are shipped/produced as fp16 (full-rate on the
PE array; halves DMA bytes); accumulation stays fp32 in PSUM. The fp32
logits are rounded to fp16 for the output DMA and upcast on the host.

v4 layout (trace-driven): the kernel is HBM-bound (~14.4 MB of traffic).
All data DMAs ride the sync HW-DGE ring in readiness order.  feat_map is
host-packed so each of 7 input triggers delivers one 2048-column block
with BOTH channel halves (8 KB/partition rows): the first key quad can
start ~1 us after the first block lands, and quad k's operands arrive
while quad k-1's output drains.  Output is staged in three full-row SBUF
buffers [*, 12800]; 18 sixth-granularity out DMAs (4 KB rows) fire as
their drains complete, keeping the ring fed from ~15 us on.  PSUM drains
are single-bank [*, 512] copies (6 main PSUM buffers -- pipeline depth
hides the cross-engine semaphore round-trip) assigned to scalar/vector
by accumulated-cost balance; warm-up matmuls un-throttle the PE HAM
clock gate before the first real matmul.

TensorE array tiling: the key projection (M=32) runs 4-way column-tiled
into one PSUM bank per quad of hw-tiles; one bias-activation drains four
tiles. The main einsum (K=32) runs 4-way row-tiled: hw-tile t keeps its
q and key_map operands on SBUF partitions 32*(t%4), so consecutive tiles
issue to distinct PE row-groups and overlap on the array.
"""

import os
import sys

sys.path.insert(0, "/opt/trn_rl_repo")
os.environ.setdefault("MYCRO_LOCAL_CACHE", "1")

import numpy as np

BATCH = 4
N_PER = 300
IN_DIM = 256
KEY_DIM = 32
FH = FW = 160
HHALF = FH // 2            # 80 rows per core
HW = HHALF * FW            # 12800 spatial positions per core
N_CORES = 8

MMN = 512                  # matmul moving free size (one fp32 PSUM bank)
N_T = HW // MMN            # 25 hw-tiles
N_BLK = 6                  # six full 2048-col blocks (quads) + one 512 tail
BLKW = 4 * MMN             # 2048 feat columns per block
# out-DMA groups: tail tile first (it drains first), then 2048-col sixths;
# the last block goes out per-512-tile so the final flush is small.
OUT_GROUPS = ((24 * 512, HW),) + tuple(
    (g * 2048, (g + 1) * 2048) for g in range(5)) + tuple(
    (c, c + 512) for c in range(10240, 12288, 512))

N_CHUNKS = ((0, 128), (128, 128), (256, 44))   # query-row chunks (300 rows)
CPACK_W = 728              # fp16: qry_wT (64) + in_featsT (600) + key_wT (64)
FPW = 2 * HW               # featP width: d0|d1 interleaved per block

_CACHE = {}


def build_nc():
    import concourse.bass as bass
    import concourse.bacc as bacc
    import concourse.mybir as mybir
    from concourse import tile

    f32 = mybir.dt.float32
    f16 = mybir.dt.float16
    Ident = mybir.ActivationFunctionType.Identity

    nc = bacc.Bacc("TRN2", target_bir_lowering=False, debug=False)

    featP = nc.dram_tensor("featP", [128, FPW], f16, kind="ExternalInput")
    cpack = nc.dram_tensor("cpack", [128, CPACK_W], f16, kind="ExternalInput")
    bpack = nc.dram_tensor("bpack", [128, 2], f32, kind="ExternalInput")
    out = nc.dram_tensor("out", [128, 3 * HW], f16, kind="ExternalOutput")

    with tile.TileContext(nc) as tc:
        with (
            tc.tile_pool(name="const", bufs=1) as cpool,
            tc.tile_pool(name="fpool", bufs=1) as fpool,
            tc.tile_pool(name="opool", bufs=1) as opool,
            tc.tile_pool(name="kmap", bufs=1) as kpool,
            tc.tile_pool(name="ps_main", bufs=7, space=bass.MemorySpace.PSUM) as ps_main,
            tc.tile_pool(name="ps_small", bufs=1, space=bass.MemorySpace.PSUM) as ps_small,
        ):
            # --- DMA ring head: consts, then the 7 paired feat blocks ----
            ct = cpool.tile([128, CPACK_W], f16, name="ct")
            nc.sync.dma_start(ct[:], cpack[:])
            bt = cpool.tile([128, 2], f32, name="bt")
            nc.sync.dma_start(bt[:], bpack[:])
            qw = (ct[:, 0:32], ct[:, 32:64])
            inT = (ct[:, 64:364], ct[:, 364:664])
            kw = (ct[:, 664:696], ct[:, 696:728])
            qb = bt[:, 0:1]        # qry_b replicated in all four bands
            kb = bt[:, 1:2]        # key_b replicated in all four bands

            # featP block k: cols [2k*BLKW, (2k+2)*BLKW) = d0 block | d1 block
            # The tiny tail block loads FIRST: it lands ~1us before block 0
            # and primes the whole matmul->drain->out pipeline.
            fp = fpool.tile([128, FPW], f16, name="fp")
            nc.sync.dma_start(fp[:, 2 * N_BLK * BLKW:FPW],
                              featP[:, 2 * N_BLK * BLKW:FPW])
            for k in range(N_BLK):
                nc.sync.dma_start(
                    fp[:, 2 * k * BLKW:2 * (k + 1) * BLKW],
                    featP[:, 2 * k * BLKW:2 * (k + 1) * BLKW],
                )

            def feat(d, t):
                # hw-tile t, channel half d -> fp column range
                k = t // 4
                if k < N_BLK:
                    c0 = 2 * k * BLKW + d * BLKW + (t % 4) * MMN
                else:
                    c0 = 2 * N_BLK * BLKW + d * MMN
                return fp[:, c0:c0 + MMN]

            # Preload the scalar-engine activation table with a dummy op at
            # t~0 (vector memsets a scratch tile first) so the ~1.3us
            # ACT_TABLE_LOAD doesn't delay the first real activation.
            warm = cpool.tile([128, 8], f32, name="warm")
            nc.vector.memset(warm[:], 0.0)
            warm16 = cpool.tile([128, 8], f16, name="warm16")
            nc.scalar.activation(warm16[:], warm[:, 0:8], Ident, bias=warm[:, 0:1])

            # --- qry projection, 4-way column-tiled (4 band copies) -------
            qp = ps_small.tile([128, MMN], f32, name="qp", tag="kp")
            for b in range(4):
                for d in range(2):
                    nc.tensor.matmul(
                        qp[32 * b:32 * b + 32, 0:N_PER],
                        qw[d],
                        inT[d],
                        start=(d == 0),
                        stop=(d == 1),
                        tile_position=(0, 32 * b),
                    )
            q_sb = cpool.tile([128, N_PER], f16, name="q_sb")
            nc.scalar.activation(q_sb[:], qp[:, 0:N_PER], Ident, bias=qb)

            # --- key_map: 4-way column-tiled, banded layout ---------------
            # hw-tile t lives on SBUF partitions 32*(t%4), columns
            # (t//4)*512; one [128,512] PSUM bank holds a whole quad and is
            # drained by a single bias-activation.
            key_map = kpool.tile([128, 7 * MMN], f16, name="key_map")

            # Drains and key-quad bias-adds are assigned to scalar/vector by
            # accumulated-cost balance (vector does bias via
            # tensor_scalar_add) so neither stream head-of-line blocks.
            acc = {"s": 0.0, "v": 0.0}

            def drain(dst, src):
                if acc["s"] + 720 < acc["v"] + 678:
                    nc.scalar.copy(dst, src)
                    acc["s"] += 720
                else:
                    nc.vector.tensor_copy(dst, src)
                    acc["v"] += 678

            def bias_add(dst, src, bias):
                if acc["s"] + 686 < acc["v"] + 678:
                    nc.scalar.activation(dst, src, Ident, bias=bias)
                    acc["s"] += 686
                else:
                    nc.vector.tensor_scalar_add(dst, src, bias)
                    acc["v"] += 678

            def key_quad(k):
                kp = ps_small.tile([128, MMN], f32, name=f"kp_{k}", tag="kp")
                nb = min(4, N_T - 4 * k)
                # d outer, band inner: each round's four column-group
                # matmuls overlap on the PE array.
                for d in range(2):
                    for b in range(nb):
                        nc.tensor.matmul(
                            kp[32 * b:32 * b + 32, :],
                            kw[d],
                            feat(d, 4 * k + b),
                            start=(d == 0),
                            stop=(d == 1),
                            tile_position=(0, 32 * b),
                        )
                p = 32 * nb
                bias_add(key_map[0:p, k * MMN:(k + 1) * MMN], kp[0:p, :],
                         kb[0:p, :])

            # --- output row-buffers: one [*, 12800] tile per query chunk --
            OB = [opool.tile([128, HW], f16, name=f"ob_{j}") for j in range(3)]

            # --- main einsum: 4-way row-tiled over band b = t%4 -----------
            # chunk-outer / tile-inner order: adjacent matmuls target
            # different PE row-groups and overlap on the array.
            def main_tiles(j, tiles):
                n0, m = N_CHUNKS[j]
                for t in tiles:
                    b = t % 4
                    kcol = (t // 4) * MMN
                    mp = ps_main.tile([128, MMN], f32, name=f"mp_{t}_{n0}", tag="mp")
                    nc.tensor.matmul(
                        mp[:m, :],
                        q_sb[32 * b:32 * b + 32, n0:n0 + m],
                        key_map[32 * b:32 * b + 32, kcol:kcol + MMN],
                        tile_position=(32 * b, 0),
                    )
                    drain(OB[j][:m, t * MMN:(t + 1) * MMN], mp[:m, :])

            # Interleave: tail quad/tile first (its data lands first), then
            # each key quad feeds its four hw-tiles.  Quads are emitted one
            # block AHEAD of their main tiles so the tensor stream never
            # stalls waiting for the current block's bias-add: while
            # bias_add(k) pends, the PE runs quad k+1's matmuls.
            key_quad(6)
            for j in range(3):
                main_tiles(j, (24,))
            for k in range(N_BLK):
                key_quad(k)
                for j in range(3):
                    main_tiles(j, range(4 * k, 4 * k + 4))

            # --- out DMAs: sixth-granularity (4KB rows), readiness order --
            for (c0, c1) in OUT_GROUPS:
                for j, (n0, m) in enumerate(N_CHUNKS):
                    nc.sync.dma_start(
                        out[0:m, j * HW + c0:j * HW + c1], OB[j][0:m, c0:c1]
                    )

    nc.compile()
    return nc


def _get_nc():
    if "nc" not in _CACHE:
        _CACHE["nc"] = build_nc()
    return _CACHE["nc"]


def make_in_maps(in_feats, feat_map, qry_w, qry_b, key_b, key_w):
    qwT = qry_w.T.astype(np.float16)                          # [256, 32]
    kwT = key_w.T.astype(np.float16)                          # [256, 32]
    bpack = np.zeros((128, 2), np.float32)
    bpack[:, 0] = np.tile(qry_b, 4)
    bpack[:, 1] = np.tile(key_b, 4)
    in_maps = []
    for c in range(N_CORES):
        b, h = divmod(c, 2)
        ifT = in_feats[b * N_PER:(b + 1) * N_PER].T.astype(np.float16)
        cpack = np.zeros((128, CPACK_W), np.float16)
        cpack[:, 0:32] = qwT[0:128]
        cpack[:, 32:64] = qwT[128:256]
        cpack[:, 64:364] = ifT[0:128]
        cpack[:, 364:664] = ifT[128:256]
        cpack[:, 664:696] = kwT[0:128]
        cpack[:, 696:728] = kwT[128:256]
        feat16 = np.ascontiguousarray(
            feat_map[b, :, h * HHALF:(h + 1) * HHALF, :]
        ).reshape(IN_DIM, HW).astype(np.float16)
        # featP: block k holds cols [2k*BLKW,(2k+2)*BLKW) = d0 cols | d1 cols
        featP = np.empty((128, FPW), np.float16)
        for k in range(N_BLK + 1):
            w = BLKW if k < N_BLK else MMN
            c0 = k * BLKW
            for d in range(2):
                featP[:, 2 * c0 + d * w:2 * c0 + (d + 1) * w] = (
                    feat16[d * 128:(d + 1) * 128, c0:c0 + w]
                )
        in_maps.append({
            "featP": featP,
            "cpack": cpack,
            "bpack": bpack,
        })
    return in_maps


def kernel(**inputs):
    in_feats = np.asarray(inputs["in_feats"], dtype=np.float32)
    feat_map = np.asarray(inputs["feat_map"], dtype=np.float32)
    qry_w = np.asarray(inputs["qry_w"], dtype=np.float32)
    qry_b = np.asarray(inputs["qry_b"], dtype=np.float32)
    key_w = np.asarray(inputs["key_w"], dtype=np.float32)
    key_b = np.asarray(inputs["key_b"], dtype=np.float32)

    from concourse import bass_utils

    nc = _get_nc()
    in_maps = make_in_maps(in_feats, feat_map, qry_w, qry_b, key_b, key_w)
    trace = os.environ.get("SEG_KERNEL_TRACE", "0") == "1"
    res = bass_utils.run_bass_kernel_spmd(
        nc, in_maps, core_ids=list(range(N_CORES)), trace=trace
    )
    _CACHE["last_result"] = res

    out = np.empty((BATCH * N_PER, FH, FW), dtype=np.float32)
    for c in range(N_CORES):
        b, h = divmod(c, 2)
        raw = res.results[c]["out"].astype(np.float32)        # [128, 3*HW]
        shard = np.empty((N_PER, HW), dtype=np.float32)
        for j, (n0, m) in enumerate(N_CHUNKS):
            shard[n0:n0 + m] = raw[0:m, j * HW:(j + 1) * HW]
        out[b * N_PER:(b + 1) * N_PER, h * HHALF:(h + 1) * HHALF, :] = (
            shard.reshape(N_PER, HHALF, FW)
        )
    return out


# revision 27
# speedup vs baseline: 1.1035x; 1.0859x over previous
"""Trainium2 Bass kernel for BaseSegHead (dynamic 1x1-conv seg logits).

Computes, for full inputs:
    qry_feats = in_feats @ qry_w.T + qry_b                  [1200, 32]
    key_map   = einsum('oc,bchw->bohw', key_w, feat_map) + key_b
    logits    = einsum('bnc,bchw->bnhw', qry_feats.reshape(4,300,32), key_map)
    out       = logits.reshape(1200, 160, 160)

Sharding: 8 cores = 4 batch images x 2 spatial (H) halves. Core c handles
batch b = c//2, rows h*80:(h+1)*80. Each core reads feat_map[b,:,rows,:],
its 300 queries, and writes a [300, 80*160] output shard -- no cross-core
communication and no duplicated feat_map reads.

Precision: matmul operands are shipped/produced as fp16 (full-rate on the
PE array; halves DMA bytes); accumulation stays fp32 in PSUM. The fp32
logits are rounded to fp16 for the output DMA and upcast on the host.

v4 layout (trace-driven): the kernel is HBM-bound (~14.4 MB of traffic).
All data DMAs ride the sync HW-DGE ring in readiness order.  feat_map is
host-packed so each of 7 input triggers delivers one 2048-column block
with BOTH channel halves (8 KB/partition rows): the first key quad can
start ~1 us after the first block lands, and quad k's operands arrive
while quad k-1's output drains.  Output is staged in three full-row SBUF
buffers [*, 12800]; 18 sixth-granularity out DMAs (4 KB rows) fire as
their drains complete, keeping the ring fed from ~15 us on.  PSUM drains
are single-bank [*, 512] copies (6 main PSUM buffers -- pipeline depth
hides the cross-engine semaphore round-trip) assigned to scalar/vector
by accumulated-cost balance; warm-up matmuls un-throttle the PE HAM
clock gate before the first real matmul.

TensorE array tiling: the key projection (M=32) runs 4-way column-tiled
into one PSUM bank per quad of hw-tiles; one bias-activation drains four
tiles. The main einsum (K=32) runs 4-way row-tiled: hw-tile t keeps its
q and key_map operands on SBUF partitions 32*(t%4), so consecutive tiles
issue to distinct PE row-groups and overlap on the array.
"""

import os
import sys

sys.path.insert(0, "/opt/trn_rl_repo")
os.environ.setdefault("MYCRO_LOCAL_CACHE", "1")

import numpy as np

BATCH = 4
N_PER = 300
IN_DIM = 256
KEY_DIM = 32
FH = FW = 160
HHALF = FH // 2            # 80 rows per core
HW = HHALF * FW            # 12800 spatial positions per core
N_CORES = 8

MMN = 512                  # matmul moving free size (one fp32 PSUM bank)
N_T = HW // MMN            # 25 hw-tiles
N_BLK = 6                  # six full 2048-col blocks (quads) + one 512 tail
BLKW = 4 * MMN             # 2048 feat columns per block
# out-DMA groups: tail tile first (it drains first), then 2048-col sixths
OUT_GROUPS = ((24 * 512, HW),) + tuple(
    (g * 2048, (g + 1) * 2048) for g in range(6))

N_CHUNKS = ((0, 128), (128, 128), (256, 44))   # query-row chunks (300 rows)
CPACK_W = 728              # fp16: qry_wT (64) + in_featsT (600) + key_wT (64)
FPW = 2 * HW               # featP width: d0|d1 interleaved per block

_CACHE = {}


def build_nc():
    import concourse.bass as bass
    import concourse.bacc as bacc
    import concourse.mybir as mybir
    from concourse import tile

    f32 = mybir.dt.float32
    f16 = mybir.dt.float16
    Ident = mybir.ActivationFunctionType.Identity

    nc = bacc.Bacc("TRN2", target_bir_lowering=False, debug=False)

    featP = nc.dram_tensor("featP", [128, FPW], f16, kind="ExternalInput")
    cpack = nc.dram_tensor("cpack", [128, CPACK_W], f16, kind="ExternalInput")
    bpack = nc.dram_tensor("bpack", [128, 2], f32, kind="ExternalInput")
    out = nc.dram_tensor("out", [128, 3 * HW], f16, kind="ExternalOutput")

    with tile.TileContext(nc) as tc:
        with (
            tc.tile_pool(name="const", bufs=1) as cpool,
            tc.tile_pool(name="fpool", bufs=1) as fpool,
            tc.tile_pool(name="opool", bufs=1) as opool,
            tc.tile_pool(name="kmap", bufs=1) as kpool,
            tc.tile_pool(name="ps_main", bufs=7, space=bass.MemorySpace.PSUM) as ps_main,
            tc.tile_pool(name="ps_small", bufs=1, space=bass.MemorySpace.PSUM) as ps_small,
        ):
            # --- DMA ring head: consts, then the 7 paired feat blocks ----
            ct = cpool.tile([128, CPACK_W], f16, name="ct")
            nc.sync.dma_start(ct[:], cpack[:])
            bt = cpool.tile([128, 2], f32, name="bt")
            nc.sync.dma_start(bt[:], bpack[:])
            qw = (ct[:, 0:32], ct[:, 32:64])
            inT = (ct[:, 64:364], ct[:, 364:664])
            kw = (ct[:, 664:696], ct[:, 696:728])
            qb = bt[:, 0:1]        # qry_b replicated in all four bands
            kb = bt[:, 1:2]        # key_b replicated in all four bands

            # featP block k: cols [2k*BLKW, (2k+2)*BLKW) = d0 block | d1 block
            # The tiny tail block loads FIRST: it lands ~1us before block 0
            # and primes the whole matmul->drain->out pipeline.
            fp = fpool.tile([128, FPW], f16, name="fp")
            nc.sync.dma_start(fp[:, 2 * N_BLK * BLKW:FPW],
                              featP[:, 2 * N_BLK * BLKW:FPW])
            for k in range(N_BLK):
                nc.sync.dma_start(
                    fp[:, 2 * k * BLKW:2 * (k + 1) * BLKW],
                    featP[:, 2 * k * BLKW:2 * (k + 1) * BLKW],
                )

            def feat(d, t):
                # hw-tile t, channel half d -> fp column range
                k = t // 4
                if k < N_BLK:
                    c0 = 2 * k * BLKW + d * BLKW + (t % 4) * MMN
                else:
                    c0 = 2 * N_BLK * BLKW + d * MMN
                return fp[:, c0:c0 + MMN]

            # Preload the scalar-engine activation table with a dummy op at
            # t~0 (vector memsets a scratch tile first) so the ~1.3us
            # ACT_TABLE_LOAD doesn't delay the first real activation.
            warm = cpool.tile([128, 8], f32, name="warm")
            nc.vector.memset(warm[:], 0.0)
            warm16 = cpool.tile([128, 8], f16, name="warm16")
            nc.scalar.activation(warm16[:], warm[:, 0:8], Ident, bias=warm[:, 0:1])

            # --- qry projection, 4-way column-tiled (4 band copies) -------
            qp = ps_small.tile([128, MMN], f32, name="qp", tag="kp")
            for b in range(4):
                for d in range(2):
                    nc.tensor.matmul(
                        qp[32 * b:32 * b + 32, 0:N_PER],
                        qw[d],
                        inT[d],
                        start=(d == 0),
                        stop=(d == 1),
                        tile_position=(0, 32 * b),
                    )
            q_sb = cpool.tile([128, N_PER], f16, name="q_sb")
            nc.scalar.activation(q_sb[:], qp[:, 0:N_PER], Ident, bias=qb)

            # --- key_map: 4-way column-tiled, banded layout ---------------
            # hw-tile t lives on SBUF partitions 32*(t%4), columns
            # (t//4)*512; one [128,512] PSUM bank holds a whole quad and is
            # drained by a single bias-activation.
            key_map = kpool.tile([128, 7 * MMN], f16, name="key_map")

            # Drains and key-quad bias-adds are assigned to scalar/vector by
            # accumulated-cost balance (vector does bias via
            # tensor_scalar_add) so neither stream head-of-line blocks.
            acc = {"s": 0.0, "v": 0.0}

            def drain(dst, src):
                if acc["s"] + 720 < acc["v"] + 678:
                    nc.scalar.copy(dst, src)
                    acc["s"] += 720
                else:
                    nc.vector.tensor_copy(dst, src)
                    acc["v"] += 678

            def bias_add(dst, src, bias):
                if acc["s"] + 686 < acc["v"] + 678:
                    nc.scalar.activation(dst, src, Ident, bias=bias)
                    acc["s"] += 686
                else:
                    nc.vector.tensor_scalar_add(dst, src, bias)
                    acc["v"] += 678

            def key_quad(k):
                kp = ps_small.tile([128, MMN], f32, name=f"kp_{k}", tag="kp")
                nb = min(4, N_T - 4 * k)
                # d outer, band inner: each round's four column-group
                # matmuls overlap on the PE array.
                for d in range(2):
                    for b in range(nb):
                        nc.tensor.matmul(
                            kp[32 * b:32 * b + 32, :],
                            kw[d],
                            feat(d, 4 * k + b),
                            start=(d == 0),
                            stop=(d == 1),
                            tile_position=(0, 32 * b),
                        )
                p = 32 * nb
                bias_add(key_map[0:p, k * MMN:(k + 1) * MMN], kp[0:p, :],
                         kb[0:p, :])

            # --- output row-buffers: one [*, 12800] tile per query chunk --
            OB = [opool.tile([128, HW], f16, name=f"ob_{j}") for j in range(3)]

            # --- main einsum: 4-way row-tiled over band b = t%4 -----------
            # chunk-outer / tile-inner order: adjacent matmuls target
            # different PE row-groups and overlap on the array.
            def main_tiles(j, tiles):
                n0, m = N_CHUNKS[j]
                for t in tiles:
                    b = t % 4
                    kcol = (t // 4) * MMN
                    mp = ps_main.tile([128, MMN], f32, name=f"mp_{t}_{n0}", tag="mp")
                    nc.tensor.matmul(
                        mp[:m, :],
                        q_sb[32 * b:32 * b + 32, n0:n0 + m],
                        key_map[32 * b:32 * b + 32, kcol:kcol + MMN],
                        tile_position=(32 * b, 0),
                    )
                    drain(OB[j][:m, t * MMN:(t + 1) * MMN], mp[:m, :])

            # Interleave: tail quad/tile first (its data lands first), then
            # each key quad feeds its four hw-tiles.  Quads are emitted one
            # block AHEAD of their main tiles so the tensor stream never
            # stalls waiting for the current block's bias-add: while
            # bias_add(k) pends, the PE runs quad k+1's matmuls.
            key_quad(6)
            for j in range(3):
                main_tiles(j, (24,))
            for k in range(N_BLK):
                key_quad(k)
                for j in range(3):
                    main_tiles(j, range(4 * k, 4 * k + 4))

            # --- out DMAs: sixth-granularity (4KB rows), readiness order --
            for (c0, c1) in OUT_GROUPS:
                for j, (n0, m) in enumerate(N_CHUNKS):
                    nc.sync.dma_start(
                        out[0:m, j * HW + c0:j * HW + c1], OB[j][0:m, c0:c1]
                    )

    nc.compile()
    return nc


def _get_nc():
    if "nc" not in _CACHE:
        _CACHE["nc"] = build_nc()
    return _CACHE["nc"]


def make_in_maps(in_feats, feat_map, qry_w, qry_b, key_b, key_w):
    qwT = qry_w.T.astype(np.float16)                          # [256, 32]
    kwT = key_w.T.astype(np.float16)                          # [256, 32]
    bpack = np.zeros((128, 2), np.float32)
    bpack[:, 0] = np.tile(qry_b, 4)
    bpack[:, 1] = np.tile(key_b, 4)
    in_maps = []
    for c in range(N_CORES):
        b, h = divmod(c, 2)
        ifT = in_feats[b * N_PER:(b + 1) * N_PER].T.astype(np.float16)
        cpack = np.zeros((128, CPACK_W), np.float16)
        cpack[:, 0:32] = qwT[0:128]
        cpack[:, 32:64] = qwT[128:256]
        cpack[:, 64:364] = ifT[0:128]
        cpack[:, 364:664] = ifT[128:256]
        cpack[:, 664:696] = kwT[0:128]
        cpack[:, 696:728] = kwT[128:256]
        feat16 = np.ascontiguousarray(
            feat_map[b, :, h * HHALF:(h + 1) * HHALF, :]
        ).reshape(IN_DIM, HW).astype(np.float16)
        # featP: block k holds cols [2k*BLKW,(2k+2)*BLKW) = d0 cols | d1 cols
        featP = np.empty((128, FPW), np.float16)
        for k in range(N_BLK + 1):
            w = BLKW if k < N_BLK else MMN
            c0 = k * BLKW
            for d in range(2):
                featP[:, 2 * c0 + d * w:2 * c0 + (d + 1) * w] = (
                    feat16[d * 128:(d + 1) * 128, c0:c0 + w]
                )
        in_maps.append({
            "featP": featP,
            "cpack": cpack,
            "bpack": bpack,
        })
    return in_maps


def kernel(**inputs):
    in_feats = np.asarray(inputs["in_feats"], dtype=np.float32)
    feat_map = np.asarray(inputs["feat_map"], dtype=np.float32)
    qry_w = np.asarray(inputs["qry_w"], dtype=np.float32)
    qry_b = np.asarray(inputs["qry_b"], dtype=np.float32)
    key_w = np.asarray(inputs["key_w"], dtype=np.float32)
    key_b = np.asarray(inputs["key_b"], dtype=np.float32)

    from concourse import bass_utils

    nc = _get_nc()
    in_maps = make_in_maps(in_feats, feat_map, qry_w, qry_b, key_b, key_w)
    trace = os.environ.get("SEG_KERNEL_TRACE", "0") == "1"
    res = bass_utils.run_bass_kernel_spmd(
        nc, in_maps, core_ids=list(range(N_CORES)), trace=trace
    )
    _CACHE["last_result"] = res

    out = np.empty((BATCH * N_PER, FH, FW), dtype=np.float32)
    for c in range(N_CORES):
        b, h = divmod(c, 2)
        raw = res.results[c]["out"].astype(np.float32)        # [128, 3*HW]
        shard = np.empty((N_PER, HW), dtype=np.float32)
        for j, (n0, m) in enumerate(N_CHUNKS):
            shard[n0:n0 + m] = raw[0:m, j * HW:(j + 1) * HW]
        out[b * N_PER:(b + 1) * N_PER, h * HHALF:(h + 1) * HHALF, :] = (
            shard.reshape(N_PER, HHALF, FW)
        )
    return out
